# revision 1
# baseline (speedup 1.0000x reference)
"""ChebNet (K=2, L=2) GNN forward on 8 Trainium2 NeuronCores.

Strategy (graph/data parallel over nodes):
  - Nodes sharded by destination: core c owns nodes [c*6250, (c+1)*6250).
  - Per layer l:  out = h @ W[l,0] + prop(h) @ W[l,1] + b
    Using (L_hat @ h) @ W1 == L_hat @ (h @ W1):
      pass1: g = h @ W[l,1]            (dense, node-major PSUM out)
      AllGather(g shards) -> g_full    (on-chip collective, separate silicon)
      pass2: per 128-dest window: PSUM += h @ W[l,0]  (dense)
                                      += S_tile.T @ gathered_g_rows  (message passing)
                                      += ones.T @ bias
             silu -> h_next; PE-transpose -> channel-major for next layer's lhsT
  - Message passing: edges sorted by destination window, 128 edges/tile.
    dma_gather fetches g_full[src] rows (2KB each); a one-hot selection
    matrix S (S[e, dest] = norm[e]) built on DVE turns segment-sum into a
    PE matmul. int16 gather indices => g_full split in two 25000-row halves.
  - All matmuls run in float32r (full PE rate, ~1.5e-4 rel err).

kernel(**inputs) takes FULL inputs, returns FULL [50000, 256] float32.
"""
import sys

sys.path.insert(0, "/opt/trn_rl_repo")
import numpy as np
from contextlib import ExitStack

import concourse.bacc as bacc
import concourse.tile as tile
import concourse.mybir as mybir
from concourse.bass_utils import run_bass_kernel_spmd
from concourse.masks import make_identity

# problem constants (hardcoded per contract)
N, E = 50000, 400000
IN, H, OUT = 256, 512, 256
L = 2
NC = 8
P = 128
NS = N // NC                # 6250 nodes per core
W = (NS + P - 1) // P       # 49 dest windows per core
HALF = N // 2               # int16 index range split
SW = 2                      # windows per gather superwindow

f32 = mybir.dt.float32
f32r = mybir.dt.float32r
i16 = mybir.dt.int16
i32 = mybir.dt.int32

_cached = {}


def _win_size(w):
    return min(P, NS - w * P)


def _node_slices():
    out = []
    a = 0
    while a < NS:
        out.append((a, min(512, NS - a)))
        a += 512
    return out


def _prep(edge_index):
    """Host-side graph preprocessing -> per-core arrays + structural program."""
    row = np.asarray(edge_index[0], dtype=np.int64)
    col = np.asarray(edge_index[1], dtype=np.int64)
    deg = np.bincount(row, minlength=N).astype(np.float32)
    with np.errstate(divide="ignore"):
        dinv = np.where(deg > 0, 1.0 / np.sqrt(deg, dtype=np.float32), 0.0).astype(
            np.float32
        )
    norm = (-(dinv[row] * dinv[col])).astype(np.float32)

    core = col // NS
    win = (col - core * NS) // P
    half = row // HALF
    # bucket edges per (core, window, half)
    key = (core * W + win) * 2 + half
    order = np.argsort(key, kind="stable")
    counts = np.bincount(key, minlength=NC * W * 2).reshape(NC, W, 2)
    starts = np.zeros((NC, W, 2), dtype=np.int64)
    starts.reshape(-1)[1:] = np.cumsum(counts.reshape(-1))[:-1]

    # structural tile counts (same on every core)
    nt = np.maximum(counts.max(axis=0) + P - 1, 0) // P  # [W, 2]

    # tile order: superwindows of SW windows; lo tiles then hi tiles
    tiles = []          # (w, h)
    calls = []          # (t_start, t_end, h, sw0) per gather call
    win_tiles = [[] for _ in range(W)]  # window -> list of global tile ids
    for sw0 in range(0, W, SW):
        ws = range(sw0, min(sw0 + SW, W))
        for h in (0, 1):
            t0 = len(tiles)
            for w in ws:
                for _ in range(nt[w, h]):
                    win_tiles[w].append(len(tiles))
                    tiles.append((w, h))
            if len(tiles) > t0:
                calls.append((t0, len(tiles), h, sw0))
    T = len(tiles)

    # per-core data arrays
    idx_all = np.zeros((NC, T, P), dtype=np.int16)
    dest_all = np.zeros((NC, T, P), dtype=np.float32)
    norm_all = np.zeros((NC, T, P), dtype=np.float32)
    src_rel = (row - half * HALF).astype(np.int64)
    dest_loc = (col - core * NS - win * P).astype(np.float32)
    # slot cursor per (core, w, h): first tile id per (w,h)
    tile_base = {}
    cur = {}
    for t, (w, h) in enumerate(tiles):
        if (w, h) not in tile_base:
            tile_base[(w, h)] = t
    for c in range(NC):
        for w in range(W):
            for h in (0, 1):
                n = counts[c, w, h]
                if n == 0:
                    continue
                eids = order[starts[c, w, h] : starts[c, w, h] + n]
                # fill consecutive slots across this (w,h)'s structural tiles
                tb = tile_base[(w, h)]
                # structural tiles for (w,h) are consecutive in global order
                flat_idx = np.zeros(nt[w, h] * P, dtype=np.int16)
                flat_dst = np.zeros(nt[w, h] * P, dtype=np.float32)
                flat_nrm = np.zeros(nt[w, h] * P, dtype=np.float32)
                flat_idx[:n] = src_rel[eids]
                flat_dst[:n] = dest_loc[eids]
                flat_nrm[:n] = norm[eids]
                idx_all[c, tb : tb + nt[w, h]] = flat_idx.reshape(-1, P)
                dest_all[c, tb : tb + nt[w, h]] = flat_dst.reshape(-1, P)
                norm_all[c, tb : tb + nt[w, h]] = flat_nrm.reshape(-1, P)

    # wrapped int16 index layout for dma_gather: [128, T*8]
    idx_wrapped = np.stack(
        [np.tile(idx_all[c].reshape(-1, 16).T, (8, 1)) for c in range(NC)]
    )  # [NC, 16->128, T*8]
    dest_sb = np.ascontiguousarray(np.transpose(dest_all, (0, 2, 1)))  # [NC,128,T]
    norm_sb = np.ascontiguousarray(np.transpose(norm_all, (0, 2, 1)))

    return dict(
        T=T,
        tiles=tiles,
        calls=calls,
        win_tiles=win_tiles,
        idx_wrapped=idx_wrapped,
        dest_sb=dest_sb,
        norm_sb=norm_sb,
        tcall_max=max(t1 - t0 for t0, t1, _, _ in calls),
    )


def _build(T, tiles, calls, win_tiles, tcall_max, sim_single=False):
    ACT = (
        mybir.ActivationFunctionType.Sigmoid
        if sim_single
        else mybir.ActivationFunctionType.Silu
    )
    nc = bacc.Bacc(
        "TRN2",
        target_bir_lowering=False,
        debug=False,
        num_devices=1 if sim_single else NC,
    )

    # ---------------- external I/O ----------------
    x_ch = nc.dram_tensor("x_ch", [IN // P, P, NS], f32r, kind="ExternalInput")
    in_w_d = nc.dram_tensor("in_w_d", [IN, H], f32r, kind="ExternalInput")
    conv_w_d = nc.dram_tensor("conv_w_d", [L, 2, H, H], f32r, kind="ExternalInput")
    out_w_d = nc.dram_tensor("out_w_d", [H, OUT], f32r, kind="ExternalInput")
    in_b_d = nc.dram_tensor("in_b_d", [H // P, P], f32, kind="ExternalInput")
    conv_b_d = nc.dram_tensor("conv_b_d", [L, H], f32r, kind="ExternalInput")
    out_b_d = nc.dram_tensor("out_b_d", [OUT // P, P], f32, kind="ExternalInput")
    idx_d = nc.dram_tensor("idx_d", [P, T * 8], i16, kind="ExternalInput")
    dest_d = nc.dram_tensor("dest_d", [P, T], f32, kind="ExternalInput")
    norm_d = nc.dram_tensor("norm_d", [P, T], f32, kind="ExternalInput")
    y = nc.dram_tensor("y", [OUT, NS], f32, kind="ExternalOutput")

    # ---------------- internal DRAM ----------------
    h_ch_a = nc.dram_tensor("h_ch_a", [W, H, P], f32r, kind="Internal")
    h_ch_b = nc.dram_tensor("h_ch_b", [W, H, P], f32r, kind="Internal")
    g_shard = nc.dram_tensor("g_shard", [NS, H], f32r, kind="Internal")
    g_full = [
        nc.dram_tensor(f"g_full{l}", [N, H], f32r, kind="Internal", addr_space="Shared")
        for l in range(L)
    ]

    KH = H // P  # 4 k-chunks of H
    nsl = _node_slices()

    with tile.TileContext(nc) as tc, ExitStack() as ctx:
        cst = ctx.enter_context(tc.tile_pool(name="cst", bufs=1))
        hwp = ctx.enter_context(tc.tile_pool(name="hwp", bufs=3))
        stg = ctx.enter_context(tc.tile_pool(name="stg", bufs=3))
        lnd = ctx.enter_context(tc.tile_pool(name="lnd", bufs=3))
        spool = ctx.enter_context(tc.tile_pool(name="spool", bufs=4))
        hnx = ctx.enter_context(tc.tile_pool(name="hnx", bufs=2))
        ps_g = ctx.enter_context(tc.tile_pool(name="ps_g", bufs=2, space="PSUM"))
        ps_o = ctx.enter_context(tc.tile_pool(name="ps_o", bufs=2, space="PSUM"))
        ps_t = ctx.enter_context(tc.tile_pool(name="ps_t", bufs=2, space="PSUM"))

        # ---------------- constants to SBUF ----------------
        in_w_sb = cst.tile([P, IN // P, KH, P], f32r, name="in_w_sb")
        nc.sync.dma_start(
            in_w_sb[:], in_w_d[:].rearrange("(k p) (m q) -> p k m q", p=P, q=P)
        )
        conv_w_sb = cst.tile([P, L, 2, KH, H], f32r, name="conv_w_sb")
        nc.sync.dma_start(
            conv_w_sb[:], conv_w_d[:].rearrange("l c (k p) n -> p l c k n", p=P)
        )
        out_w_sb = cst.tile([P, KH, OUT // P, P], f32r, name="out_w_sb")
        nc.sync.dma_start(
            out_w_sb[:], out_w_d[:].rearrange("(k p) (m q) -> p k m q", p=P, q=P)
        )
        in_b_sb = cst.tile([P, H // P], f32, name="in_b_sb")
        nc.sync.dma_start(in_b_sb[:], in_b_d[:].rearrange("m p -> p m"))
        conv_b_sb = cst.tile([1, L, H], f32r, name="conv_b_sb")
        nc.sync.dma_start(conv_b_sb[:], conv_b_d[:].rearrange("(o l) n -> o l n", o=1))
        out_b_sb = cst.tile([P, OUT // P], f32, name="out_b_sb")
        nc.sync.dma_start(out_b_sb[:], out_b_d[:].rearrange("m p -> p m"))
        idx_sb = cst.tile([P, T * 8], i16, name="idx_sb")
        nc.sync.dma_start(idx_sb[:], idx_d[:])
        dest_sb = cst.tile([P, T], f32, name="dest_sb")
        nc.sync.dma_start(dest_sb[:], dest_d[:])
        norm_sb = cst.tile([P, T], f32, name="norm_sb")
        nc.sync.dma_start(norm_sb[:], norm_d[:])

        iota_i = cst.tile([P, P], i32, name="iota_i")
        nc.gpsimd.iota(iota_i[:], pattern=[[1, P]], base=0, channel_multiplier=0)
        iota_f = cst.tile([P, P], f32, name="iota_f")
        nc.vector.tensor_copy(iota_f[:], iota_i[:])
        ident_f = cst.tile([P, P], f32, name="ident_f")
        make_identity(nc, ident_f[:])
        ident = cst.tile([P, P], f32r, name="ident")
        nc.vector.tensor_copy(ident[:], ident_f[:])
        ones_f = cst.tile([1, P], f32, name="ones_f")
        nc.vector.memset(ones_f[:], 1.0)
        ones_r = cst.tile([1, P], f32r, name="ones_r")
        nc.vector.tensor_copy(ones_r[:], ones_f[:])

        # ---------------- input layer: h0 = silu(x @ in_w + in_b), ch-major ----
        for si, (a, ln) in enumerate(nsl):
            xsb = hwp.tile([P, IN // P, 512], f32r, name="xsb")
            nc.sync.dma_start(
                xsb[:, :, :ln], x_ch[:, :, a : a + ln].rearrange("k p n -> p k n")
            )
            for m in range(KH):
                pg = ps_g.tile([P, 512], f32, name="pg")
                for k in range(IN // P):
                    nc.tensor.matmul(
                        pg[:, :ln],
                        in_w_sb[:, k, m, :],
                        xsb[:, k, :ln],
                        start=(k == 0),
                        stop=(k == IN // P - 1),
                    )
                hsb = stg.tile([P, 512], f32r, name="hsb")
                nc.scalar.activation(
                    hsb[:, :ln],
                    pg[:, :ln],
                    ACT,
                    bias=in_b_sb[:, m : m + 1],
                )
                for j in range((ln + P - 1) // P):
                    w = (a + j * P) // P
                    wl = _win_size(w)
                    nc.sync.dma_start(
                        h_ch_a[w, m * P : (m + 1) * P, :wl],
                        hsb[:, j * P : j * P + wl],
                    )

        h_cur, h_nxt = h_ch_a, h_ch_b
        # ---------------- ChebConv layers ----------------
        for l in range(L):
            # pass 1: g = h @ conv_w[l, 1]  (node-major out)
            for w in range(W):
                wl = _win_size(w)
                hw = hwp.tile([P, KH, P], f32r, name="hw1")
                nc.sync.dma_start(
                    hw[:], h_cur[w].rearrange("(k p) n -> p k n", p=P)
                )
                pg = ps_g.tile([P, 512], f32, name="pg")
                for k in range(KH):
                    nc.tensor.matmul(
                        pg[:],
                        hw[:, k, :],
                        conv_w_sb[:, l, 1, k, :],
                        start=(k == 0),
                        stop=(k == KH - 1),
                    )
                gst = stg.tile([P, 512], f32r, name="gst")
                nc.vector.tensor_copy(gst[:], pg[:])
                nc.sync.dma_start(g_shard[w * P : w * P + wl, :], gst[:wl, :])

            if sim_single:
                # single-core sim stand-in: place own shard at slot 0
                nc.sync.dma_start(g_full[l][0:NS, :], g_shard[:])
            else:
                nc.gpsimd.collective_compute(
                    "AllGather",
                    mybir.AluOpType.bypass,
                    replica_groups=[list(range(NC))],
                    ins=[g_shard[:].opt()],
                    outs=[g_full[l][:].opt()],
                )
            g_lo = g_full[l][0:HALF, :]
            g_hi = g_full[l][HALF:N, :]

            # pass 2: per superwindow gather, per window accumulate
            land_of_call = {}
            for sw0 in range(0, W, SW):
                ws = list(range(sw0, min(sw0 + SW, W)))
                # issue gather calls for this superwindow
                for t0, t1, h, s0 in calls:
                    if s0 != sw0:
                        continue
                    nt_call = t1 - t0
                    land = lnd.tile([P, tcall_max, H], f32r, name="land")
                    nc.gpsimd.dma_gather(
                        land[:, :nt_call, :],
                        g_lo if h == 0 else g_hi,
                        idx_sb[:, 8 * t0 : 8 * t1],
                        nt_call * P,
                        nt_call * P,
                        H,
                        single_packet=False,
                    )
                    for t in range(t0, t1):
                        land_of_call[t] = (land, t - t0)
                for w in ws:
                    wl = _win_size(w)
                    hw = hwp.tile([P, KH, P], f32r, name="hw2")
                    nc.sync.dma_start(
                        hw[:], h_cur[w].rearrange("(k p) n -> p k n", p=P)
                    )
                    po = ps_o.tile([P, 512], f32, name="po")
                    for k in range(KH):
                        nc.tensor.matmul(
                            po[:],
                            hw[:, k, :],
                            conv_w_sb[:, l, 0, k, :],
                            start=(k == 0),
                            stop=False,
                        )
                    wt = win_tiles[w]
                    nc.tensor.matmul(
                        po[:],
                        ones_r[:1, :],
                        conv_b_sb[:1, l, :],
                        start=False,
                        stop=(not wt),
                    )
                    for i, t in enumerate(wt):
                        s_t = spool.tile([P, P], f32r, name="s_t")
                        nc.vector.tensor_scalar(
                            s_t[:],
                            iota_f[:],
                            dest_sb[:, t : t + 1],
                            norm_sb[:, t : t + 1],
                            op0=mybir.AluOpType.is_equal,
                            op1=mybir.AluOpType.mult,
                        )
                        land, rel = land_of_call[t]
                        nc.tensor.matmul(
                            po[:],
                            s_t[:],
                            land[:, rel, :],
                            start=False,
                            stop=(i == len(wt) - 1),
                        )
                    hn = hnx.tile([P, 512], f32r, name="hn")
                    nc.scalar.activation(hn[:], po[:], ACT)
                    pt = ps_t.tile([P, 512], f32r, name="pt")
                    for k in range(KH):
                        nc.tensor.transpose(
                            pt[:, k * P : (k + 1) * P], hn[:, k * P : (k + 1) * P], ident[:]
                        )
                    tst = stg.tile([P, 512], f32r, name="tst")
                    nc.vector.tensor_copy(tst[:], pt[:])
                    nc.sync.dma_start(
                        h_nxt[w].rearrange("(k p) n -> p k n", p=P)[:, :, :wl],
                        tst[:].rearrange("p (k n) -> p k n", k=KH)[:, :, :wl],
                    )
            h_cur, h_nxt = h_nxt, h_cur

        # ---------------- output layer: y = h2 @ out_w + out_b (ch-major out) --
        for m in range(OUT // P):
            for si, (a, ln) in enumerate(nsl):
                wb = a // P
                nw = (ln + P - 1) // P
                pg = ps_g.tile([P, 512], f32, name="pg")
                for k in range(KH):
                    rhs = hwp.tile([P, 4, P], f32r, name="rhs_o")
                    nc.sync.dma_start(
                        rhs[:, :nw, :],
                        h_cur[wb : wb + nw, k * P : (k + 1) * P, :].rearrange(
                            "w p n -> p w n"
                        ),
                    )
                    nc.tensor.matmul(
                        pg[:, :ln],
                        out_w_sb[:, k, m, :],
                        rhs[:, :nw, :].rearrange("p w n -> p (w n)")[:, :ln],
                        start=(k == 0),
                        stop=(k == KH - 1),
                    )
                ysb = stg.tile([P, 512], f32, name="ysb")
                nc.scalar.activation(
                    ysb[:, :ln],
                    pg[:, :ln],
                    mybir.ActivationFunctionType.Identity,
                    bias=out_b_sb[:, m : m + 1],
                )
                nc.sync.dma_start(y[m * P : (m + 1) * P, a : a + ln], ysb[:, :ln])

    nc.compile()
    return nc


def _get_nc_and_prep(edge_index):
    key = "k"
    if key not in _cached:
        prep = _prep(edge_index)
        nc = _build(
            prep["T"], prep["tiles"], prep["calls"], prep["win_tiles"], prep["tcall_max"]
        )
        _cached[key] = (nc, prep)
    return _cached[key]


def kernel(x, edge_index, in_w, in_b, conv_w, conv_b, out_w, out_b, trace=False):
    x = np.asarray(x, dtype=np.float32)
    in_w = np.ascontiguousarray(np.asarray(in_w, dtype=np.float32))
    in_b = np.asarray(in_b, dtype=np.float32)
    conv_w = np.ascontiguousarray(np.asarray(conv_w, dtype=np.float32))
    conv_b = np.ascontiguousarray(np.asarray(conv_b, dtype=np.float32))
    out_w = np.ascontiguousarray(np.asarray(out_w, dtype=np.float32))
    out_b = np.asarray(out_b, dtype=np.float32)

    nc, prep = _get_nc_and_prep(edge_index)

    in_b_r = np.ascontiguousarray(in_b.reshape(H // P, P))
    out_b_r = np.ascontiguousarray(out_b.reshape(OUT // P, P))
    in_maps = []
    for c in range(NC):
        xs = np.ascontiguousarray(
            x[c * NS : (c + 1) * NS].T.reshape(IN // P, P, NS)
        )
        in_maps.append(
            dict(
                x_ch=xs,
                in_w_d=in_w,
                conv_w_d=conv_w,
                out_w_d=out_w,
                in_b_d=in_b_r,
                conv_b_d=conv_b,
                out_b_d=out_b_r,
                idx_d=np.ascontiguousarray(prep["idx_wrapped"][c]),
                dest_d=prep["dest_sb"][c],
                norm_d=prep["norm_sb"][c],
            )
        )

    res = run_bass_kernel_spmd(nc, in_maps, core_ids=list(range(NC)), trace=trace)
    out = np.concatenate([res.results[c]["y"].T for c in range(NC)], axis=0)
    kernel.last_exec_time_ns = res.exec_time_ns
    kernel.last_results = res
    return out


if __name__ == "__main__":
    rng = np.random.default_rng(0)
    ei = rng.integers(0, N, size=(2, E)).astype(np.int64)
    p = _prep(ei)
    print("T =", p["T"], "tcall_max =", p["tcall_max"], "ncalls =", len(p["calls"]))



# revision 3
# speedup vs baseline: 10.6440x; 10.6440x over previous
"""ChebNet (K=2, L=2) GNN forward on 8 Trainium2 NeuronCores.

Device kernel (graph/data parallel over nodes), same structure as the
validated baseline:
  - Nodes sharded by destination: core c owns nodes [c*6250, (c+1)*6250).
  - Per layer l:  out = h @ W[l,0] + prop(h) @ W[l,1] + b
    Using (L_hat @ h) @ W1 == L_hat @ (h @ W1):
      pass1: g = h @ W[l,1]            (dense, node-major PSUM out)
      AllGather(g shards) -> g_full    (on-chip collective)
      pass2: per 128-dest window: PSUM += h @ W[l,0]  (dense)
                                      += S_tile.T @ gathered_g_rows
                                      += ones.T @ bias
             silu -> h_next; PE-transpose -> channel-major for next layer
  - Message passing: edges sorted by destination window, 128 edges/tile.
    dma_gather fetches g_full[src] rows; a one-hot selection matrix S
    (S[e, dest] = norm[e]) turns segment-sum into a PE matmul.

Host orchestration (the axon tunnel is ~60-80 MB/s only with many
parallel small transfers, so transfers dominate wall time):
  - The sharded executable is AOT-compiled once (fast-dispatch, no
    effects) and cached across kernel() calls.
  - Weights + graph tables are uploaded once and kept device-resident;
    per call they are byte-compared against the cached host copies and
    only re-uploaded when they actually change.
  - Per call only x goes up and y comes down, both as bf16 (half the
    bytes; ~4e-3 rel err vs the 2e-2 gate), as 8/16 parallel per-shard
    transfers.
  - The donated output buffers (required so NeuronCC writes in place)
    are recycled from the previous call's outputs.

kernel(**inputs) takes FULL inputs, returns FULL [50000, 256] float32.
"""
import sys

sys.path.insert(0, "/opt/trn_rl_repo")
import numpy as np
import ml_dtypes
from concurrent.futures import ThreadPoolExecutor
from contextlib import ExitStack

import jax
from jax.sharding import Mesh, NamedSharding, PartitionSpec

try:  # jax moved shard_map out of experimental
    from jax.experimental.shard_map import shard_map
except ImportError:  # pragma: no cover
    from jax.shard_map import shard_map

import concourse.bacc as bacc
import concourse.tile as tile
import concourse.mybir as mybir
from concourse.bass2jax import (
    _bass_exec_p,
    partition_id_tensor,
    install_neuronx_cc_hook,
    fast_dispatch_compile,
)
from concourse.masks import make_identity

# problem constants (hardcoded per contract)
N, E = 50000, 400000
IN, H, OUT = 256, 512, 256
L = 2
NC = 8
P = 128
NS = N // NC                # 6250 nodes per core
W = (NS + P - 1) // P       # 49 dest windows per core
HALF = N // 2               # int16 index range split
SW = 2                      # windows per gather superwindow

f32 = mybir.dt.float32
f32r = mybir.dt.float32r
bf16 = mybir.dt.bfloat16
i16 = mybir.dt.int16
i32 = mybir.dt.int32

BF16 = ml_dtypes.bfloat16

_state = {}


def _win_size(w):
    return min(P, NS - w * P)


def _node_slices():
    out = []
    a = 0
    while a < NS:
        out.append((a, min(512, NS - a)))
        a += 512
    return out


def _prep(edge_index):
    """Host-side graph preprocessing -> per-core arrays + structural program."""
    row = np.asarray(edge_index[0], dtype=np.int64)
    col = np.asarray(edge_index[1], dtype=np.int64)
    deg = np.bincount(row, minlength=N).astype(np.float32)
    with np.errstate(divide="ignore"):
        dinv = np.where(deg > 0, 1.0 / np.sqrt(deg, dtype=np.float32), 0.0).astype(
            np.float32
        )
    norm = (-(dinv[row] * dinv[col])).astype(np.float32)

    core = col // NS
    win = (col - core * NS) // P
    half = row // HALF
    # bucket edges per (core, window, half)
    key = (core * W + win) * 2 + half
    order = np.argsort(key, kind="stable")
    counts = np.bincount(key, minlength=NC * W * 2).reshape(NC, W, 2)
    starts = np.zeros((NC, W, 2), dtype=np.int64)
    starts.reshape(-1)[1:] = np.cumsum(counts.reshape(-1))[:-1]

    # structural tile counts (same on every core)
    nt = np.maximum(counts.max(axis=0) + P - 1, 0) // P  # [W, 2]

    # tile order: superwindows of SW windows; lo tiles then hi tiles
    tiles = []          # (w, h)
    calls = []          # (t_start, t_end, h, sw0) per gather call
    win_tiles = [[] for _ in range(W)]  # window -> list of global tile ids
    for sw0 in range(0, W, SW):
        ws = range(sw0, min(sw0 + SW, W))
        for h in (0, 1):
            t0 = len(tiles)
            for w in ws:
                for _ in range(nt[w, h]):
                    win_tiles[w].append(len(tiles))
                    tiles.append((w, h))
            if len(tiles) > t0:
                calls.append((t0, len(tiles), h, sw0))
    T = len(tiles)

    # per-core data arrays
    idx_all = np.zeros((NC, T, P), dtype=np.int16)
    dest_all = np.zeros((NC, T, P), dtype=np.float32)
    norm_all = np.zeros((NC, T, P), dtype=np.float32)
    src_rel = (row - half * HALF).astype(np.int64)
    dest_loc = (col - core * NS - win * P).astype(np.float32)
    tile_base = {}
    for t, (w, h) in enumerate(tiles):
        if (w, h) not in tile_base:
            tile_base[(w, h)] = t
    for c in range(NC):
        for w in range(W):
            for h in (0, 1):
                n = counts[c, w, h]
                if n == 0:
                    continue
                eids = order[starts[c, w, h] : starts[c, w, h] + n]
                tb = tile_base[(w, h)]
                flat_idx = np.zeros(nt[w, h] * P, dtype=np.int16)
                flat_dst = np.zeros(nt[w, h] * P, dtype=np.float32)
                flat_nrm = np.zeros(nt[w, h] * P, dtype=np.float32)
                flat_idx[:n] = src_rel[eids]
                flat_dst[:n] = dest_loc[eids]
                flat_nrm[:n] = norm[eids]
                idx_all[c, tb : tb + nt[w, h]] = flat_idx.reshape(-1, P)
                dest_all[c, tb : tb + nt[w, h]] = flat_dst.reshape(-1, P)
                norm_all[c, tb : tb + nt[w, h]] = flat_nrm.reshape(-1, P)

    # wrapped int16 index layout for dma_gather: [128, T*8]
    idx_wrapped = np.stack(
        [np.tile(idx_all[c].reshape(-1, 16).T, (8, 1)) for c in range(NC)]
    )  # [NC, 16->128, T*8]
    dest_sb = np.ascontiguousarray(np.transpose(dest_all, (0, 2, 1)))  # [NC,128,T]
    norm_sb = np.ascontiguousarray(np.transpose(norm_all, (0, 2, 1)))

    return dict(
        T=T,
        tiles=tiles,
        calls=calls,
        win_tiles=win_tiles,
        idx_wrapped=idx_wrapped,
        dest_sb=dest_sb,
        norm_sb=norm_sb,
        tcall_max=max(t1 - t0 for t0, t1, _, _ in calls),
    )


def _build(T, tiles, calls, win_tiles, tcall_max):
    ACT = mybir.ActivationFunctionType.Silu
    nc = bacc.Bacc(
        "TRN2",
        target_bir_lowering=False,
        debug=False,
        num_devices=NC,
    )

    # ---------------- external I/O ----------------
    x_ch = nc.dram_tensor("x_ch", [IN // P, P, NS], bf16, kind="ExternalInput")
    in_w_d = nc.dram_tensor("in_w_d", [IN, H], f32r, kind="ExternalInput")
    conv_w_d = nc.dram_tensor("conv_w_d", [L, 2, H, H], f32r, kind="ExternalInput")
    out_w_d = nc.dram_tensor("out_w_d", [H, OUT], f32r, kind="ExternalInput")
    in_b_d = nc.dram_tensor("in_b_d", [H // P, P], f32, kind="ExternalInput")
    conv_b_d = nc.dram_tensor("conv_b_d", [L, H], f32r, kind="ExternalInput")
    out_b_d = nc.dram_tensor("out_b_d", [OUT // P, P], f32, kind="ExternalInput")
    idx_d = nc.dram_tensor("idx_d", [P, T * 8], i16, kind="ExternalInput")
    dest_d = nc.dram_tensor("dest_d", [P, T], f32, kind="ExternalInput")
    norm_d = nc.dram_tensor("norm_d", [P, T], f32, kind="ExternalInput")
    # two channel-half outputs -> 16 parallel fetches host-side
    y0 = nc.dram_tensor("y0", [P, NS], bf16, kind="ExternalOutput")
    y1 = nc.dram_tensor("y1", [P, NS], bf16, kind="ExternalOutput")
    y_out = [y0, y1]

    # ---------------- internal DRAM ----------------
    h_ch_a = nc.dram_tensor("h_ch_a", [W, H, P], f32r, kind="Internal")
    h_ch_b = nc.dram_tensor("h_ch_b", [W, H, P], f32r, kind="Internal")
    g_shard = nc.dram_tensor("g_shard", [NS, H], f32r, kind="Internal")
    g_full = [
        nc.dram_tensor(f"g_full{l}", [N, H], f32r, kind="Internal", addr_space="Shared")
        for l in range(L)
    ]

    KH = H // P  # 4 k-chunks of H
    nsl = _node_slices()

    with tile.TileContext(nc) as tc, ExitStack() as ctx:
        cst = ctx.enter_context(tc.tile_pool(name="cst", bufs=1))
        hwp = ctx.enter_context(tc.tile_pool(name="hwp", bufs=3))
        stg = ctx.enter_context(tc.tile_pool(name="stg", bufs=3))
        lnd = ctx.enter_context(tc.tile_pool(name="lnd", bufs=3))
        spool = ctx.enter_context(tc.tile_pool(name="spool", bufs=4))
        hnx = ctx.enter_context(tc.tile_pool(name="hnx", bufs=2))
        ps_g = ctx.enter_context(tc.tile_pool(name="ps_g", bufs=2, space="PSUM"))
        ps_o = ctx.enter_context(tc.tile_pool(name="ps_o", bufs=2, space="PSUM"))
        ps_t = ctx.enter_context(tc.tile_pool(name="ps_t", bufs=2, space="PSUM"))

        # ---------------- constants to SBUF ----------------
        in_w_sb = cst.tile([P, IN // P, KH, P], f32r, name="in_w_sb")
        nc.sync.dma_start(
            in_w_sb[:], in_w_d[:].rearrange("(k p) (m q) -> p k m q", p=P, q=P)
        )
        conv_w_sb = cst.tile([P, L, 2, KH, H], f32r, name="conv_w_sb")
        nc.sync.dma_start(
            conv_w_sb[:], conv_w_d[:].rearrange("l c (k p) n -> p l c k n", p=P)
        )
        out_w_sb = cst.tile([P, KH, OUT // P, P], f32r, name="out_w_sb")
        nc.sync.dma_start(
            out_w_sb[:], out_w_d[:].rearrange("(k p) (m q) -> p k m q", p=P, q=P)
        )
        in_b_sb = cst.tile([P, H // P], f32, name="in_b_sb")
        nc.sync.dma_start(in_b_sb[:], in_b_d[:].rearrange("m p -> p m"))
        conv_b_sb = cst.tile([1, L, H], f32r, name="conv_b_sb")
        nc.sync.dma_start(conv_b_sb[:], conv_b_d[:].rearrange("(o l) n -> o l n", o=1))
        out_b_sb = cst.tile([P, OUT // P], f32, name="out_b_sb")
        nc.sync.dma_start(out_b_sb[:], out_b_d[:].rearrange("m p -> p m"))
        idx_sb = cst.tile([P, T * 8], i16, name="idx_sb")
        nc.sync.dma_start(idx_sb[:], idx_d[:])
        dest_sb = cst.tile([P, T], f32, name="dest_sb")
        nc.sync.dma_start(dest_sb[:], dest_d[:])
        norm_sb = cst.tile([P, T], f32, name="norm_sb")
        nc.sync.dma_start(norm_sb[:], norm_d[:])

        iota_i = cst.tile([P, P], i32, name="iota_i")
        nc.gpsimd.iota(iota_i[:], pattern=[[1, P]], base=0, channel_multiplier=0)
        iota_f = cst.tile([P, P], f32, name="iota_f")
        nc.vector.tensor_copy(iota_f[:], iota_i[:])
        ident_f = cst.tile([P, P], f32, name="ident_f")
        make_identity(nc, ident_f[:])
        ident = cst.tile([P, P], f32r, name="ident")
        nc.vector.tensor_copy(ident[:], ident_f[:])
        ones_f = cst.tile([1, P], f32, name="ones_f")
        nc.vector.memset(ones_f[:], 1.0)
        ones_r = cst.tile([1, P], f32r, name="ones_r")
        nc.vector.tensor_copy(ones_r[:], ones_f[:])

        # ---------------- input layer: h0 = silu(x @ in_w + in_b), ch-major ----
        for si, (a, ln) in enumerate(nsl):
            xsb_b = hwp.tile([P, IN // P, 512], bf16, name="xsb_b")
            nc.sync.dma_start(
                xsb_b[:, :, :ln], x_ch[:, :, a : a + ln].rearrange("k p n -> p k n")
            )
            xsb = hwp.tile([P, IN // P, 512], f32r, name="xsb")
            nc.vector.tensor_copy(xsb[:, :, :ln], xsb_b[:, :, :ln])
            for m in range(KH):
                pg = ps_g.tile([P, 512], f32, name="pg")
                for k in range(IN // P):
                    nc.tensor.matmul(
                        pg[:, :ln],
                        in_w_sb[:, k, m, :],
                        xsb[:, k, :ln],
                        start=(k == 0),
                        stop=(k == IN // P - 1),
                    )
                hsb = stg.tile([P, 512], f32r, name="hsb")
                nc.scalar.activation(
                    hsb[:, :ln],
                    pg[:, :ln],
                    ACT,
                    bias=in_b_sb[:, m : m + 1],
                )
                for j in range((ln + P - 1) // P):
                    w = (a + j * P) // P
                    wl = _win_size(w)
                    nc.sync.dma_start(
                        h_ch_a[w, m * P : (m + 1) * P, :wl],
                        hsb[:, j * P : j * P + wl],
                    )

        h_cur, h_nxt = h_ch_a, h_ch_b
        # ---------------- ChebConv layers ----------------
        for l in range(L):
            # pass 1: g = h @ conv_w[l, 1]  (node-major out)
            for w in range(W):
                wl = _win_size(w)
                hw = hwp.tile([P, KH, P], f32r, name="hw1")
                nc.sync.dma_start(
                    hw[:], h_cur[w].rearrange("(k p) n -> p k n", p=P)
                )
                pg = ps_g.tile([P, 512], f32, name="pg")
                for k in range(KH):
                    nc.tensor.matmul(
                        pg[:],
                        hw[:, k, :],
                        conv_w_sb[:, l, 1, k, :],
                        start=(k == 0),
                        stop=(k == KH - 1),
                    )
                gst = stg.tile([P, 512], f32r, name="gst")
                nc.vector.tensor_copy(gst[:], pg[:])
                nc.sync.dma_start(g_shard[w * P : w * P + wl, :], gst[:wl, :])

            nc.gpsimd.collective_compute(
                "AllGather",
                mybir.AluOpType.bypass,
                replica_groups=[list(range(NC))],
                ins=[g_shard[:].opt()],
                outs=[g_full[l][:].opt()],
            )
            g_lo = g_full[l][0:HALF, :]
            g_hi = g_full[l][HALF:N, :]

            # pass 2: per superwindow gather, per window accumulate
            land_of_call = {}
            for sw0 in range(0, W, SW):
                ws = list(range(sw0, min(sw0 + SW, W)))
                for t0, t1, h, s0 in calls:
                    if s0 != sw0:
                        continue
                    nt_call = t1 - t0
                    land = lnd.tile([P, tcall_max, H], f32r, name="land")
                    nc.gpsimd.dma_gather(
                        land[:, :nt_call, :],
                        g_lo if h == 0 else g_hi,
                        idx_sb[:, 8 * t0 : 8 * t1],
                        nt_call * P,
                        nt_call * P,
                        H,
                        single_packet=False,
                    )
                    for t in range(t0, t1):
                        land_of_call[t] = (land, t - t0)
                for w in ws:
                    wl = _win_size(w)
                    hw = hwp.tile([P, KH, P], f32r, name="hw2")
                    nc.sync.dma_start(
                        hw[:], h_cur[w].rearrange("(k p) n -> p k n", p=P)
                    )
                    po = ps_o.tile([P, 512], f32, name="po")
                    for k in range(KH):
                        nc.tensor.matmul(
                            po[:],
                            hw[:, k, :],
                            conv_w_sb[:, l, 0, k, :],
                            start=(k == 0),
                            stop=False,
                        )
                    wt = win_tiles[w]
                    nc.tensor.matmul(
                        po[:],
                        ones_r[:1, :],
                        conv_b_sb[:1, l, :],
                        start=False,
                        stop=(not wt),
                    )
                    for i, t in enumerate(wt):
                        s_t = spool.tile([P, P], f32r, name="s_t")
                        nc.vector.tensor_scalar(
                            s_t[:],
                            iota_f[:],
                            dest_sb[:, t : t + 1],
                            norm_sb[:, t : t + 1],
                            op0=mybir.AluOpType.is_equal,
                            op1=mybir.AluOpType.mult,
                        )
                        land, rel = land_of_call[t]
                        nc.tensor.matmul(
                            po[:],
                            s_t[:],
                            land[:, rel, :],
                            start=False,
                            stop=(i == len(wt) - 1),
                        )
                    hn = hnx.tile([P, 512], f32r, name="hn")
                    nc.scalar.activation(hn[:], po[:], ACT)
                    pt = ps_t.tile([P, 512], f32r, name="pt")
                    for k in range(KH):
                        nc.tensor.transpose(
                            pt[:, k * P : (k + 1) * P], hn[:, k * P : (k + 1) * P], ident[:]
                        )
                    tst = stg.tile([P, 512], f32r, name="tst")
                    nc.vector.tensor_copy(tst[:], pt[:])
                    nc.sync.dma_start(
                        h_nxt[w].rearrange("(k p) n -> p k n", p=P)[:, :, :wl],
                        tst[:].rearrange("p (k n) -> p k n", k=KH)[:, :, :wl],
                    )
            h_cur, h_nxt = h_nxt, h_cur

        # ---------------- output layer: y = h2 @ out_w + out_b (ch-major out) --
        for m in range(OUT // P):
            for si, (a, ln) in enumerate(nsl):
                wb = a // P
                nw = (ln + P - 1) // P
                pg = ps_g.tile([P, 512], f32, name="pg")
                for k in range(KH):
                    rhs = hwp.tile([P, 4, P], f32r, name="rhs_o")
                    nc.sync.dma_start(
                        rhs[:, :nw, :],
                        h_cur[wb : wb + nw, k * P : (k + 1) * P, :].rearrange(
                            "w p n -> p w n"
                        ),
                    )
                    nc.tensor.matmul(
                        pg[:, :ln],
                        out_w_sb[:, k, m, :],
                        rhs[:, :nw, :].rearrange("p w n -> p (w n)")[:, :ln],
                        start=(k == 0),
                        stop=(k == KH - 1),
                    )
                ysb = stg.tile([P, 512], f32, name="ysb")
                nc.scalar.activation(
                    ysb[:, :ln],
                    pg[:, :ln],
                    mybir.ActivationFunctionType.Identity,
                    bias=out_b_sb[:, m : m + 1],
                )
                yb = stg.tile([P, 512], bf16, name="yb")
                nc.vector.tensor_copy(yb[:, :ln], ysb[:, :ln])
                nc.sync.dma_start(y_out[m][:, a : a + ln], yb[:, :ln])

    nc.compile()
    return nc


# ---------------------------------------------------------------------------
# host orchestration
# ---------------------------------------------------------------------------

STATIC_NAMES = (
    "in_w_d",
    "conv_w_d",
    "out_w_d",
    "in_b_d",
    "conv_b_d",
    "out_b_d",
    "idx_d",
    "dest_d",
    "norm_d",
)


def _static_host_arrays(prep, in_w, in_b, conv_w, conv_b, out_w, out_b):
    """name -> list of per-core host arrays (weights replicated)."""
    in_b_r = np.ascontiguousarray(in_b.reshape(H // P, P))
    out_b_r = np.ascontiguousarray(out_b.reshape(OUT // P, P))
    rep = lambda a: [a] * NC
    return {
        "in_w_d": rep(in_w),
        "conv_w_d": rep(conv_w),
        "out_w_d": rep(out_w),
        "in_b_d": rep(in_b_r),
        "conv_b_d": rep(conv_b),
        "out_b_d": rep(out_b_r),
        "idx_d": [np.ascontiguousarray(prep["idx_wrapped"][c]) for c in range(NC)],
        "dest_d": [prep["dest_sb"][c] for c in range(NC)],
        "norm_d": [prep["norm_sb"][c] for c in range(NC)],
    }


def _put_sharded(per_core_arrays, sharding, devices, pool):
    """Upload per-core shards in parallel, assemble a global jax.Array."""
    futs = [
        pool.submit(jax.device_put, per_core_arrays[c], devices[c])
        for c in range(NC)
    ]
    shards = [f.result() for f in futs]
    s0 = per_core_arrays[0].shape
    global_shape = (NC * s0[0],) + tuple(s0[1:])
    return jax.make_array_from_single_device_arrays(global_shape, sharding, shards)


def _setup(edge_index, in_w, in_b, conv_w, conv_b, out_w, out_b):
    install_neuronx_cc_hook()
    prep = _prep(edge_index)
    nc = _build(
        prep["T"], prep["tiles"], prep["calls"], prep["win_tiles"], prep["tcall_max"]
    )

    partition_name = nc.partition_id_tensor.name if nc.partition_id_tensor else None
    in_names, out_names, out_avals = [], [], []
    per_core_shapes = {}
    for alloc in nc.m.functions[0].allocations:
        if not isinstance(alloc, mybir.MemoryLocationSet):
            continue
        name = alloc.memorylocations[0].name
        if alloc.kind == "ExternalInput":
            if name != partition_name:
                in_names.append(name)
                per_core_shapes[name] = (
                    tuple(alloc.tensor_shape),
                    mybir.dt.np(alloc.dtype),
                )
        elif alloc.kind == "ExternalOutput":
            out_names.append(name)
            out_avals.append(
                jax.core.ShapedArray(tuple(alloc.tensor_shape), mybir.dt.np(alloc.dtype))
            )
    assert in_names[0] == "x_ch" and tuple(in_names[1:]) == STATIC_NAMES, in_names
    n_params = len(in_names)
    n_outs = len(out_names)
    all_in_names = list(in_names) + list(out_names)
    if partition_name is not None:
        all_in_names.append(partition_name)

    def _body(*args):
        operands = list(args)
        if partition_name is not None:
            operands.append(partition_id_tensor())
        outs = _bass_exec_p.bind(
            *operands,
            out_avals=tuple(out_avals),
            in_names=tuple(all_in_names),
            out_names=tuple(out_names),
            lowering_input_output_aliases=(),
            sim_require_finite=True,
            sim_require_nnan=True,
            nc=nc,
        )
        return tuple(outs)

    devices = jax.devices()[:NC]
    mesh = Mesh(np.asarray(devices), ("core",))
    sh = NamedSharding(mesh, PartitionSpec("core"))
    in_specs = (PartitionSpec("core"),) * (n_params + n_outs)
    out_specs = (PartitionSpec("core"),) * n_outs
    donate = tuple(range(n_params, n_params + n_outs))

    arg_structs = []
    for name in in_names:
        shape, dtype = per_core_shapes[name]
        arg_structs.append(
            jax.ShapeDtypeStruct((NC * shape[0],) + tuple(shape[1:]), dtype, sharding=sh)
        )
    for av in out_avals:
        arg_structs.append(
            jax.ShapeDtypeStruct((NC * av.shape[0],) + tuple(av.shape[1:]), av.dtype, sharding=sh)
        )

    def compile_fn():
        jitted = jax.jit(
            shard_map(
                _body, mesh=mesh, in_specs=in_specs, out_specs=out_specs, check_rep=False
            ),
            donate_argnums=donate,
            keep_unused=True,
        )
        return jitted.lower(*arg_structs).compile()

    compiled = fast_dispatch_compile(compile_fn)

    pool = ThreadPoolExecutor(max_workers=16)

    statics_host = _static_host_arrays(prep, in_w, in_b, conv_w, conv_b, out_w, out_b)
    statics_dev = {
        name: _put_sharded(
            [np.ascontiguousarray(a) for a in statics_host[name]], sh, devices, pool
        )
        for name in STATIC_NAMES
    }

    # initial donation buffers (contents irrelevant: kernel writes all of y)
    donate_bufs = []
    for av in out_avals:
        z = np.zeros(av.shape, av.dtype)
        donate_bufs.append(_put_sharded([z] * NC, sh, devices, pool))

    _state.update(
        nc=nc,
        prep=prep,
        compiled=compiled,
        devices=devices,
        sharding=sh,
        pool=pool,
        in_names=in_names,
        out_names=out_names,
        statics_dev=statics_dev,
        donate_bufs=donate_bufs,
        # host copies for change detection
        ref=dict(
            edge_index=np.array(edge_index, copy=True),
            in_w=np.array(in_w, copy=True),
            in_b=np.array(in_b, copy=True),
            conv_w=np.array(conv_w, copy=True),
            conv_b=np.array(conv_b, copy=True),
            out_w=np.array(out_w, copy=True),
            out_b=np.array(out_b, copy=True),
        ),
    )
    return _state


def _ensure(edge_index, in_w, in_b, conv_w, conv_b, out_w, out_b):
    if not _state:
        return _setup(edge_index, in_w, in_b, conv_w, conv_b, out_w, out_b)
    ref = _state["ref"]
    if not np.array_equal(ref["edge_index"], edge_index):
        # graph changed: full rebuild (different structural program)
        _state.clear()
        return _setup(edge_index, in_w, in_b, conv_w, conv_b, out_w, out_b)
    cur = dict(in_w=in_w, in_b=in_b, conv_w=conv_w, conv_b=conv_b,
               out_w=out_w, out_b=out_b)
    if any(not np.array_equal(ref[k], cur[k]) for k in cur):
        # weights changed: re-upload statics only
        statics_host = _static_host_arrays(
            _state["prep"], in_w, in_b, conv_w, conv_b, out_w, out_b
        )
        _state["statics_dev"] = {
            name: _put_sharded(
                [np.ascontiguousarray(a) for a in statics_host[name]],
                _state["sharding"], _state["devices"], _state["pool"],
            )
            for name in STATIC_NAMES
        }
        for k in cur:
            ref[k] = np.array(cur[k], copy=True)
    return _state


def _upload_x(x, st):
    """x [N, IN] f32 -> sharded bf16 channel-major [NC*(IN//P), P, NS]."""
    devices, pool = st["devices"], st["pool"]

    def conv_put(c):
        xs = np.ascontiguousarray(
            x[c * NS : (c + 1) * NS].astype(BF16).T
        ).reshape(IN // P, P, NS)
        return jax.device_put(xs, devices[c])

    futs = [pool.submit(conv_put, c) for c in range(NC)]
    shards = [f.result() for f in futs]
    return jax.make_array_from_single_device_arrays(
        (NC * (IN // P), P, NS), st["sharding"], shards
    )


def kernel(x, edge_index, in_w, in_b, conv_w, conv_b, out_w, out_b, trace=False):
    x = np.asarray(x, dtype=np.float32)
    in_w = np.ascontiguousarray(np.asarray(in_w, dtype=np.float32))
    in_b = np.asarray(in_b, dtype=np.float32)
    conv_w = np.ascontiguousarray(np.asarray(conv_w, dtype=np.float32))
    conv_b = np.ascontiguousarray(np.asarray(conv_b, dtype=np.float32))
    out_w = np.ascontiguousarray(np.asarray(out_w, dtype=np.float32))
    out_b = np.asarray(out_b, dtype=np.float32)

    st = _ensure(edge_index, in_w, in_b, conv_w, conv_b, out_w, out_b)

    xg = _upload_x(x, st)
    args = [xg] + [st["statics_dev"][name] for name in st["in_names"][1:]]
    outs = st["compiled"](*args, *st["donate_bufs"])
    st["donate_bufs"] = list(outs)  # recycle as next call's donation buffers

    # fetch y halves: 16 parallel 1.6MB transfers, convert per shard
    out = np.empty((N, OUT), np.float32)
    pool = st["pool"]

    def fetch(m, shard):
        c = shard.index[0].start // P if shard.index[0].start else 0
        # shard covers global rows [c*P, (c+1)*P) of y_m -> channels m*P..
        data = np.asarray(shard.data)  # [P, NS] bf16
        out[c * NS : (c + 1) * NS, m * P : (m + 1) * P] = data.T.astype(np.float32)

    futs = []
    for m, yg in enumerate(outs):
        for shard in yg.addressable_shards:
            futs.append(pool.submit(fetch, m, shard))
    for f in futs:
        f.result()

    kernel.last_exec_time_ns = None
    return out


kernel.last_exec_time_ns = None


if __name__ == "__main__":
    rng = np.random.default_rng(0)
    ei = rng.integers(0, N, size=(2, E)).astype(np.int64)
    p = _prep(ei)
    print("T =", p["T"], "tcall_max =", p["tcall_max"], "ncalls =", len(p["calls"]))
